# revision 8
# baseline (speedup 1.0000x reference)
"""3D Haar DWT (clean-mode subband stack) on 8 Trainium2 NeuronCores.

Problem (hardcoded): inputs (4, 128, 128, 128, 4) f32, A (128, 128) f32 Haar
analysis operator. Output (4, 64, 64, 64, 32) f32 = 8 subbands stacked on the
channel axis (LLL, LLH, LHL, LHH, HLL, HLH, HHL, HHH) x 4 channels.

Sharding: pure data parallel over (batch, d1-half): core k handles
b = k // 2, d1 range [64*(k%2), 64*(k%2)+64). The Haar transform is a 2-tap
non-overlapping filter (rows of A touch only columns 2i, 2i+1), so splitting
d1 on an even boundary requires no communication.

The whole device data path runs in bf16 (the rel-err budget is 2e-2; bf16
end-to-end lands ~5e-3), which halves both HBM streams vs f32 — the kernel
is memory-bound, so this is the dominant win. Host casts f32 -> bf16 while
staging the per-core slabs and upcasts the bf16 result; only device/HBM
traffic is on the critical path.

Per-core pipeline (slab pre-transposed on host to [d2, ci, t, d1, m, c]
where d3 = 2m + t, i.e. even/odd d3 planes pre-split per chunk so every DVE
access is a contiguous bf16 run):
  1. DMA in 1 MiB chunks (8 d1 slices), partitions = d2, 8 KiB descriptors.
  2. d3 butterfly on DVE: two whole-chunk contiguous tensor_tensor ops
     (bf16 2x packed mode).
  3. d2 transform as PE matmul (stationary bf16 +-0.5*A^T, FWL fast path),
     with the d1 butterfly folded into PSUM accumulation; 8 matmuls fill a
     4-bank [128, 2048] PSUM tile per d1-pair-pair "group".
  4. One PSUM -> SBUF evacuation op per group (FD=2048, pure copy + f32 ->
     bf16 cast; all scaling lives in the weights), alternating between the
     scalar (ACT) and vector (DVE) engines to balance their load.
  5. One 1 MiB DMA out per chunk on SWDGE (8 KiB descriptors; SWDGE so
     stores never head-of-line-block the load queue); host reassembles the
     subband-major layout.

Scale bookkeeping: reference applies A three times (factor s = 1/sqrt(2) per
nonzero). The d3/d1 butterflies apply +/-1 and the matmul applies
0.5*A = (0.5*s)*sign-pattern, so each path gets s^3 exactly as the
reference (the 0.5 supplies the two butterflies' missing s each).
"""

import sys

import numpy as np

if "/opt/trn_rl_repo" not in sys.path:
    sys.path.insert(0, "/opt/trn_rl_repo")

B, N, C = 4, 128, 4
N_CORES = 8
SLAB = 64          # d1 extent per core
D1C = 8            # d1 values per chunk
NCHUNK = SLAB // D1C
PAIRS = D1C // 2   # d1 pairs per chunk
GROUPS = PAIRS // 2  # 2 d1-pairs per PSUM group
MC = (N // 2) * C  # 256: contiguous (m, c) run per d3 parity plane

_BASS_CACHE = {}


def _haar_matrix():
    s = np.float32(1.0 / np.sqrt(2.0))
    A = np.zeros((N, N), dtype=np.float32)
    for i in range(N // 2):
        A[i, 2 * i] = s
        A[i, 2 * i + 1] = s
        A[64 + i, 2 * i] = -s
        A[64 + i, 2 * i + 1] = s
    return A


def _reference_numpy(inputs, A):
    # Fallback only: exact reference math on host (used if A is not Haar).
    x = np.einsum("ij,bpjqc->bpiqc", A, inputs)
    x = np.einsum("ij,bjpqc->bipqc", A, x)
    x = np.einsum("ij,bpqjc->bpqic", A, x)
    m = x.shape[1] // 2
    subs = [
        x[:, :m, :m, :m, :], x[:, :m, :m, m:, :],
        x[:, :m, m:, :m, :], x[:, :m, m:, m:, :],
        x[:, m:, :m, :m, :], x[:, m:, :m, m:, :],
        x[:, m:, m:, :m, :], x[:, m:, m:, m:, :],
    ]
    return np.concatenate(subs, axis=-1).astype(np.float32)


def _build_bass():
    import concourse.bacc as bacc
    import concourse.mybir as mybir
    import concourse.tile as tile

    f32 = mybir.dt.float32
    bf16 = mybir.dt.bfloat16

    # Bacc (not raw Bass): its compile() pipeline splits multi-sem waits into
    # EventSemaphore instructions — TRN2 instructions have one wait slot.
    nc = bacc.Bacc("TRN2", target_bir_lowering=False, debug=False)
    # x is host-pre-transposed to [d2, chunk, t, d1, m*c] (d3 = 2m + t) so
    # each load descriptor is an 8 KiB contiguous run per partition and the
    # d3 butterfly is two fully-contiguous ops per chunk.
    x = nc.dram_tensor("x", [N, NCHUNK, 2, D1C, MC], bf16,
                       kind="ExternalInput")
    atp = nc.dram_tensor("atp", [N, N], bf16, kind="ExternalInput")
    atn = nc.dram_tensor("atn", [N, N], bf16, kind="ExternalInput")
    # y dims: (i2, chunk, s1, pp, s3*o3*c); i2 = s2*64 + o2. One store per
    # chunk: an 8 KiB contiguous run per partition.
    y = nc.dram_tensor("y", [N, NCHUNK, 2, PAIRS, 2 * MC], bf16,
                       kind="ExternalOutput")

    with tile.TileContext(nc) as tc:
        with (
            tc.tile_pool(name="const", bufs=1) as cpool,
            tc.tile_pool(name="io", bufs=6) as tpool,
            tc.tile_pool(name="mid", bufs=3) as mpool,
            tc.tile_pool(name="psum", bufs=2, space="PSUM") as ppool,
        ):
            atp_sb = cpool.tile([N, N], bf16)
            atn_sb = cpool.tile([N, N], bf16)

            # Deferred work from odd chunks: the DVE evacuation (and the
            # store that must follow it — Tile dependencies follow emission
            # order) is emitted only after the NEXT chunk's butterflies, so
            # the DVE FIFO never stalls a butterfly behind a PSUM-dependent
            # cast.
            pending = []

            def flush_pending():
                for dst, src, store_out, store_in in pending:
                    nc.vector.tensor_copy(out=dst, in_=src)
                    nc.gpsimd.dma_start(out=store_out, in_=store_in)
                pending.clear()
            for ci in range(NCHUNK):
                # 1. load chunk: [d2 | t, d1, m*c] — one 1 MiB DMA,
                # 128 descriptors of 8 KiB.
                T = tpool.tile([N, 2, D1C, MC], bf16, tag="T")
                nc.sync.dma_start(out=T[:], in_=x[:, ci])
                if ci == 0:
                    # consts after the first bulk load so the data pipeline
                    # starts immediately
                    nc.sync.dma_start(out=atp_sb[:], in_=atp[:, :])
                    nc.sync.dma_start(out=atn_sb[:], in_=atn[:, :])

                # 2. d3 butterfly: W[:, 0] = even+odd (low),
                # W[:, 1] = odd-even (high). Fully contiguous bf16 runs ->
                # DVE 2x packed mode.
                W = mpool.tile([N, 2, D1C, MC], bf16, tag="W")
                nc.vector.tensor_add(out=W[:, 0], in0=T[:, 0], in1=T[:, 1])
                nc.vector.tensor_sub(out=W[:, 1], in0=T[:, 1], in1=T[:, 0])

                flush_pending()

                # staging: (s1, o1_local, s3*o3*c)
                Yst = mpool.tile([N, 2, PAIRS, 2 * MC], bf16, tag="Yst")

                for g in range(GROUPS):
                    # rhs views for the two d1 pairs of this group; free
                    # order (k=s3, m, c) matches the subband split layout.
                    r00 = W[:, :, 4 * g + 0]
                    r01 = W[:, :, 4 * g + 1]
                    r10 = W[:, :, 4 * g + 2]
                    r11 = W[:, :, 4 * g + 3]
                    # One 4-bank PSUM tile per group:
                    # [lo(p0) | lo(p1) | hi(p0) | hi(p1)]
                    ps = ppool.tile([N, 4 * 2 * MC], f32, tag="ps")
                    # 3. d2 transform + d1 butterfly in PSUM. atp runs
                    # first (6 matmuls), then atn (2) — 2 weight loads per
                    # group.
                    mm = nc.tensor.matmul
                    mm(ps[:, 0 * 512:1 * 512], lhsT=atp_sb[:], rhs=r00,
                       start=True, stop=False)
                    mm(ps[:, 0 * 512:1 * 512], lhsT=atp_sb[:], rhs=r01,
                       start=False, stop=True)
                    mm(ps[:, 1 * 512:2 * 512], lhsT=atp_sb[:], rhs=r10,
                       start=True, stop=False)
                    mm(ps[:, 1 * 512:2 * 512], lhsT=atp_sb[:], rhs=r11,
                       start=False, stop=True)
                    mm(ps[:, 2 * 512:3 * 512], lhsT=atp_sb[:], rhs=r01,
                       start=True, stop=False)
                    mm(ps[:, 3 * 512:4 * 512], lhsT=atp_sb[:], rhs=r11,
                       start=True, stop=False)
                    mm(ps[:, 2 * 512:3 * 512], lhsT=atn_sb[:], rhs=r00,
                       start=False, stop=True)
                    mm(ps[:, 3 * 512:4 * 512], lhsT=atn_sb[:], rhs=r10,
                       start=False, stop=True)
                    # 4. one evacuation op per group: psum layout
                    # (s1, pair-in-group, s3*m*c) matches the Yst slice.
                    # ACT : DVE = 12 : 4 — DVE also owns the butterflies;
                    # DVE casts are deferred past the next chunk's
                    # butterflies to keep the DVE FIFO dense.
                    dst = Yst[:, :, 2 * g:2 * g + 2]
                    src = ps[:].rearrange("p (a q f) -> p a q f", a=2, q=2)
                    if g == 1 and ci % 2 == 1:
                        pending.append((dst, src, y[:, ci], Yst[:]))
                    else:
                        nc.scalar.copy(dst, src)

                # 5. one store per chunk on SWDGE (gpsimd) so stores never
                # head-of-line-block the load queue on the SP sequencer.
                # (odd chunks: store rides with the deferred DVE cast)
                if ci % 2 == 0:
                    nc.gpsimd.dma_start(out=y[:, ci], in_=Yst[:])

            flush_pending()
    nc.compile()
    return nc


def make_in_maps(x, A):
    """Stage per-core inputs: transpose slab to [d2, ci, t, d1, m, c]
    (d1 = 8*ci + d1_local, d3 = 2m + t) and cast to bf16; weights are
    +-0.5*A^T in bf16."""
    import ml_dtypes

    atp = np.ascontiguousarray((0.5 * A.T).astype(ml_dtypes.bfloat16))
    atn = np.ascontiguousarray((-0.5 * A.T).astype(ml_dtypes.bfloat16))
    in_maps = []
    for k in range(N_CORES):
        b, h = divmod(k, 2)
        slab = x[b, h * SLAB:(h + 1) * SLAB]          # [d1, d2, d3, c]
        pre = (
            slab.transpose(1, 0, 2, 3)                # [d2, d1, d3, c]
            .reshape(N, NCHUNK, D1C, N // 2, 2, C)    # [d2, ci, d1c, m, t, c]
            .transpose(0, 1, 4, 2, 3, 5)              # [d2, ci, t, d1c, m, c]
            .reshape(N, NCHUNK, 2, D1C, MC)
        )
        in_maps.append(
            {
                "x": np.ascontiguousarray(pre.astype(ml_dtypes.bfloat16)),
                "atp": atp,
                "atn": atn,
            }
        )
    return in_maps


def assemble_out(results):
    """Reassemble per-core y buffers [i2, ci, s1, pp, s3, o3, c] (bf16) into
    the full (B, 64, 64, 64, 32) f32 output."""
    out = np.empty((B, 64, 64, 64, 8 * C), np.float32)
    for k in range(N_CORES):
        b, h = divmod(k, 2)
        arr = results[k]["y"].astype(np.float32).reshape(
            2, 64, NCHUNK, 2, PAIRS, 2, 64, C
        )  # (s2, o2, ci, s1, pp, s3, o3, c)
        out[b, 32 * h:32 * h + 32] = (
            arr.transpose(2, 4, 1, 6, 3, 0, 5, 7)  # (ci, pp, o2, o3, s1, s2, s3, c)
            .reshape(32, 64, 64, 8 * C)
        )
    return out


def kernel(**inputs):
    x = np.ascontiguousarray(np.asarray(inputs["inputs"], dtype=np.float32))
    A = np.asarray(inputs["A"], dtype=np.float32)
    assert x.shape == (B, N, N, N, C), x.shape

    if not np.allclose(A, _haar_matrix(), atol=1e-5):
        # Kernel hardcodes the 2-tap Haar structure; fall back for generic A.
        return _reference_numpy(x, A)

    from concourse.bass_utils import run_bass_kernel_spmd

    if "nc" not in _BASS_CACHE:
        _BASS_CACHE["nc"] = _build_bass()
    nc = _BASS_CACHE["nc"]

    in_maps = make_in_maps(x, A)
    res = run_bass_kernel_spmd(nc, in_maps, core_ids=list(range(N_CORES)))
    return assemble_out(res.results)


# revision 10
# speedup vs baseline: 1.1308x; 1.1308x over previous
"""3D Haar DWT (clean-mode subband stack) on 8 Trainium2 NeuronCores.

Problem (hardcoded): inputs (4, 128, 128, 128, 4) f32, A (128, 128) f32 Haar
analysis operator. Output (4, 64, 64, 64, 32) f32 = 8 subbands stacked on the
channel axis (LLL, LLH, LHL, LHH, HLL, HLH, HHL, HHH) x 4 channels.

Sharding: pure data parallel over (batch, d1-half): core k handles
b = k // 2, d1 range [64*(k%2), 64*(k%2)+64). The Haar transform is a 2-tap
non-overlapping filter (rows of A touch only columns 2i, 2i+1), so splitting
d1 on an even boundary requires no communication.

The whole device data path runs in bf16 (the rel-err budget is 2e-2; bf16
end-to-end lands ~5e-3), which halves both HBM streams vs f32 — the kernel
is memory-bound, so this is the dominant win. Host casts f32 -> bf16 while
staging the per-core slabs and upcasts the bf16 result; only device/HBM
traffic is on the critical path.

Per-core pipeline (slab pre-transposed on host to [d2, ci, t, d1, m, c]
where d3 = 2m + t, i.e. even/odd d3 planes pre-split per chunk so every DVE
access is a contiguous bf16 run):
  1. DMA in 1 MiB chunks (8 d1 slices), partitions = d2, 8 KiB descriptors.
  2. d3 butterfly on DVE: two whole-chunk contiguous tensor_tensor ops
     (bf16 2x packed mode).
  3. d2 transform as PE matmul (stationary bf16 +-0.5*A^T, FWL fast path),
     with the d1 butterfly folded into PSUM accumulation; 8 matmuls fill a
     4-bank [128, 2048] PSUM tile per d1-pair-pair "group".
  4. One PSUM -> SBUF evacuation op per group (FD=2048, pure copy + f32 ->
     bf16 cast; all scaling lives in the weights), alternating between the
     scalar (ACT) and vector (DVE) engines to balance their load.
  5. One 1 MiB DMA out per chunk on SWDGE (8 KiB descriptors; SWDGE so
     stores never head-of-line-block the load queue); host reassembles the
     subband-major layout.

Scale bookkeeping: reference applies A three times (factor s = 1/sqrt(2) per
nonzero). The d3/d1 butterflies apply +/-1 and the matmul applies
0.5*A = (0.5*s)*sign-pattern, so each path gets s^3 exactly as the
reference (the 0.5 supplies the two butterflies' missing s each).
"""

import sys

import numpy as np

if "/opt/trn_rl_repo" not in sys.path:
    sys.path.insert(0, "/opt/trn_rl_repo")

B, N, C = 4, 128, 4
N_CORES = 8
SLAB = 64          # d1 extent per core
D1C = 8            # d1 values per chunk
NCHUNK = SLAB // D1C
PAIRS = D1C // 2   # d1 pairs per chunk
GROUPS = PAIRS // 2  # 2 d1-pairs per PSUM group
MC = (N // 2) * C  # 256: contiguous (m, c) run per d3 parity plane

_BASS_CACHE = {}


def _haar_matrix():
    s = np.float32(1.0 / np.sqrt(2.0))
    A = np.zeros((N, N), dtype=np.float32)
    for i in range(N // 2):
        A[i, 2 * i] = s
        A[i, 2 * i + 1] = s
        A[64 + i, 2 * i] = -s
        A[64 + i, 2 * i + 1] = s
    return A


def _reference_numpy(inputs, A):
    # Fallback only: exact reference math on host (used if A is not Haar).
    x = np.einsum("ij,bpjqc->bpiqc", A, inputs)
    x = np.einsum("ij,bjpqc->bipqc", A, x)
    x = np.einsum("ij,bpqjc->bpqic", A, x)
    m = x.shape[1] // 2
    subs = [
        x[:, :m, :m, :m, :], x[:, :m, :m, m:, :],
        x[:, :m, m:, :m, :], x[:, :m, m:, m:, :],
        x[:, m:, :m, :m, :], x[:, m:, :m, m:, :],
        x[:, m:, m:, :m, :], x[:, m:, m:, m:, :],
    ]
    return np.concatenate(subs, axis=-1).astype(np.float32)


def _build_bass():
    import concourse.bacc as bacc
    import concourse.mybir as mybir
    import concourse.tile as tile

    f32 = mybir.dt.float32
    bf16 = mybir.dt.bfloat16

    # Bacc (not raw Bass): its compile() pipeline splits multi-sem waits into
    # EventSemaphore instructions — TRN2 instructions have one wait slot.
    nc = bacc.Bacc("TRN2", target_bir_lowering=False, debug=False)
    # x is host-pre-transposed to [d2, chunk, t, d1, m*c] (d3 = 2m + t) so
    # each load descriptor is an 8 KiB contiguous run per partition and the
    # d3 butterfly is two fully-contiguous ops per chunk.
    x = nc.dram_tensor("x", [N, NCHUNK, 2, D1C, MC], bf16,
                       kind="ExternalInput")
    atp = nc.dram_tensor("atp", [N, N], bf16, kind="ExternalInput")
    atn = nc.dram_tensor("atn", [N, N], bf16, kind="ExternalInput")
    # y dims: (i2, chunk, s1, pp, s3*o3*c); i2 = s2*64 + o2. One store per
    # chunk: an 8 KiB contiguous run per partition.
    y = nc.dram_tensor("y", [N, NCHUNK, 2, PAIRS, 2 * MC], bf16,
                       kind="ExternalOutput")

    with tile.TileContext(nc) as tc:
        with (
            tc.tile_pool(name="const", bufs=1) as cpool,
            tc.tile_pool(name="io", bufs=6) as tpool,
            tc.tile_pool(name="mid", bufs=4) as mpool,
            tc.tile_pool(name="psum", bufs=4, space="PSUM") as ppool,
        ):
            atp_sb = cpool.tile([N, N], bf16)
            atn_sb = cpool.tile([N, N], bf16)

            # Deferred work from odd chunks: the DVE evacuation (and the
            # store that must follow it — Tile dependencies follow emission
            # order) is emitted only after the NEXT chunk's butterflies, so
            # the DVE FIFO never stalls a butterfly behind a PSUM-dependent
            # cast.
            pending = []

            def flush_pending():
                for dst, src, store_out, store_in in pending:
                    nc.vector.tensor_copy(out=dst, in_=src)
                    nc.gpsimd.dma_start(out=store_out, in_=store_in)
                pending.clear()
            for ci in range(NCHUNK):
                # 1. load chunk: [d2 | t, d1, m*c] — one 1 MiB DMA,
                # 128 descriptors of 8 KiB.
                T = tpool.tile([N, 2, D1C, MC], bf16, tag="T")
                nc.sync.dma_start(out=T[:], in_=x[:, ci])
                if ci == 0:
                    # consts after the first bulk load so the data pipeline
                    # starts immediately
                    nc.sync.dma_start(out=atp_sb[:], in_=atp[:, :])
                    nc.sync.dma_start(out=atn_sb[:], in_=atn[:, :])

                # 2. d3 butterfly: W[:, 0] = even+odd (low),
                # W[:, 1] = odd-even (high). Fully contiguous bf16 runs ->
                # DVE 2x packed mode.
                W = mpool.tile([N, 2, D1C, MC], bf16, tag="W")
                nc.vector.tensor_add(out=W[:, 0], in0=T[:, 0], in1=T[:, 1])
                nc.vector.tensor_sub(out=W[:, 1], in0=T[:, 1], in1=T[:, 0])

                flush_pending()

                # staging: (s1, o1_local, s3*o3*c)
                Yst = mpool.tile([N, 2, PAIRS, 2 * MC], bf16, tag="Yst")

                for pp in range(PAIRS):
                    # rhs views for this d1 pair; free order (k=s3, m, c)
                    # matches the subband split layout.
                    r0 = W[:, :, 2 * pp + 0]
                    r1 = W[:, :, 2 * pp + 1]
                    # One 2-bank PSUM tile per pair: [lo | hi]. Small tiles
                    # keep the MM -> evac -> reuse loop shorter than the
                    # chunk cadence (PSUM is the scarce resource).
                    ps = ppool.tile([N, 2 * 2 * MC], f32, tag="ps")
                    # 3. d2 transform + d1 butterfly in PSUM. atp runs
                    # first (3 matmuls), then atn — 2 weight loads per pair.
                    mm = nc.tensor.matmul
                    mm(ps[:, 0 * 512:1 * 512], lhsT=atp_sb[:], rhs=r0,
                       start=True, stop=False)
                    mm(ps[:, 0 * 512:1 * 512], lhsT=atp_sb[:], rhs=r1,
                       start=False, stop=True)
                    mm(ps[:, 1 * 512:2 * 512], lhsT=atp_sb[:], rhs=r1,
                       start=True, stop=False)
                    mm(ps[:, 1 * 512:2 * 512], lhsT=atn_sb[:], rhs=r0,
                       start=False, stop=True)
                    # 4. one evacuation op per pair: psum layout (s1,
                    # s3*m*c) matches the Yst slice. ACT takes 3 pairs per
                    # chunk, DVE the last — DVE casts are deferred past the
                    # next chunk's butterflies to keep the DVE FIFO dense
                    # (the chunk's store rides along, since Tile
                    # dependencies follow emission order).
                    dst = Yst[:, :, pp]
                    src = ps[:].rearrange("p (a f) -> p a f", a=2)
                    if pp == PAIRS - 1:
                        pending.append((dst, src, y[:, ci], Yst[:]))
                    else:
                        nc.scalar.copy(dst, src)

            flush_pending()
    nc.compile()
    return nc


def make_in_maps(x, A):
    """Stage per-core inputs: transpose slab to [d2, ci, t, d1, m, c]
    (d1 = 8*ci + d1_local, d3 = 2m + t) and cast to bf16; weights are
    +-0.5*A^T in bf16."""
    import ml_dtypes

    atp = np.ascontiguousarray((0.5 * A.T).astype(ml_dtypes.bfloat16))
    atn = np.ascontiguousarray((-0.5 * A.T).astype(ml_dtypes.bfloat16))
    in_maps = []
    for k in range(N_CORES):
        b, h = divmod(k, 2)
        slab = x[b, h * SLAB:(h + 1) * SLAB]          # [d1, d2, d3, c]
        pre = (
            slab.transpose(1, 0, 2, 3)                # [d2, d1, d3, c]
            .reshape(N, NCHUNK, D1C, N // 2, 2, C)    # [d2, ci, d1c, m, t, c]
            .transpose(0, 1, 4, 2, 3, 5)              # [d2, ci, t, d1c, m, c]
            .reshape(N, NCHUNK, 2, D1C, MC)
        )
        in_maps.append(
            {
                "x": np.ascontiguousarray(pre.astype(ml_dtypes.bfloat16)),
                "atp": atp,
                "atn": atn,
            }
        )
    return in_maps


def assemble_out(results):
    """Reassemble per-core y buffers [i2, ci, s1, pp, s3, o3, c] (bf16) into
    the full (B, 64, 64, 64, 32) f32 output."""
    out = np.empty((B, 64, 64, 64, 8 * C), np.float32)
    for k in range(N_CORES):
        b, h = divmod(k, 2)
        arr = results[k]["y"].astype(np.float32).reshape(
            2, 64, NCHUNK, 2, PAIRS, 2, 64, C
        )  # (s2, o2, ci, s1, pp, s3, o3, c)
        out[b, 32 * h:32 * h + 32] = (
            arr.transpose(2, 4, 1, 6, 3, 0, 5, 7)  # (ci, pp, o2, o3, s1, s2, s3, c)
            .reshape(32, 64, 64, 8 * C)
        )
    return out


def kernel(**inputs):
    x = np.ascontiguousarray(np.asarray(inputs["inputs"], dtype=np.float32))
    A = np.asarray(inputs["A"], dtype=np.float32)
    assert x.shape == (B, N, N, N, C), x.shape

    if not np.allclose(A, _haar_matrix(), atol=1e-5):
        # Kernel hardcodes the 2-tap Haar structure; fall back for generic A.
        return _reference_numpy(x, A)

    from concourse.bass_utils import run_bass_kernel_spmd

    if "nc" not in _BASS_CACHE:
        _BASS_CACHE["nc"] = _build_bass()
    nc = _BASS_CACHE["nc"]

    in_maps = make_in_maps(x, A)
    res = run_bass_kernel_spmd(nc, in_maps, core_ids=list(range(N_CORES)))
    return assemble_out(res.results)
